# revision 11
# baseline (speedup 1.0000x reference)
# Trainium2 Bass kernel for nn_EquivariantTransposeConv.
# Self-contained: hardcodes shapes (B=4, H=W=128, R=4, C=22) and the sharding
# (8 cores, each core = half of one image = 256 HR rows).
#
# Per-core pipeline (all planar: channels on partitions, pixels on free dim):
#   1. host: pack LR input into a 4-quarter replicated, row-sheared planar
#      tensor xin[128, 19, 130] (quarter q rows 64q..64q+64 of the core's
#      256-row slab), plus small weight tensors derived from runtime inputs.
#   2. upsample (depthwise 6x6 stride-4 transpose conv) via per-(phase,tap)
#      tensor_scalar ops -> feat bf16 [128, 34, 516] per 32-row chunk.
#   3. ctx 3x3 depthwise conv via tensor_scalar/tensor_tensor -> ctx bf16.
#   4. per output row (512 px): replicate x/y channels to 484 product rows
#      with two 0/1 matmuls per 128-row chunk, multiply on VectorE, contract
#      with the folded Wigner tensor via 4 accumulating matmuls -> out[22,512].
#   5. host: gather per-core planar outputs, transpose to (B, N, 22), fix the
#      1-pixel border of each image exactly in fp32 numpy (the kernel computes
#      garbage there since edge-replication of ctx is not done on-device).
import math
import os
import sys
from fractions import Fraction

import numpy as np
import ml_dtypes

sys.path.insert(0, "/opt/trn_rl_repo")

C_FEAT = 22
R = 4
TP_K = 6
KS = 3
B, H, W = 4, 128, 128
Hr, Wr = H * R, W * R
PAD_TOP = 1
PAD_L = 1
N_XIN_ROWS = 19
N_XIN_COLS = 130
NCHUNK = 4          # z-row chunks
ZROWS = 128         # z-rows per chunk
BF16 = ml_dtypes.bfloat16

# ---------------------------------------------------------------------------
# Wigner 3j tables (identical math to the reference, self-contained copy)
# ---------------------------------------------------------------------------

def _fact(n):
    return Fraction(math.factorial(round(n)), 1)

def _su2_cg_coeff(j1, m1, j2, m2, j3, m3):
    if m3 != m1 + m2:
        return 0.0
    vmin = int(max(-j1 + j2 + m3, -j1 + m1, 0))
    vmax = int(min(j2 + j3 + m1, j3 - j1 + j2, j3 + m3))
    c = float((2.0 * j3 + 1.0) * Fraction(
        _fact(j3 + j1 - j2) * _fact(j3 - j1 + j2) * _fact(j1 + j2 - j3) * _fact(j3 + m3) * _fact(j3 - m3),
        _fact(j1 + j2 + j3 + 1) * _fact(j1 - m1) * _fact(j1 + m1) * _fact(j2 - m2) * _fact(j2 + m2))) ** 0.5
    s = 0.0
    for v in range(vmin, vmax + 1):
        s += (-1.0) ** int(v + j2 + m2) * float(Fraction(
            _fact(j2 + j3 + m1 - v) * _fact(j1 - m1 + v),
            _fact(v) * _fact(j3 - j1 + j2 - v) * _fact(j3 + m3 - v) * _fact(v + j1 - j2 - m3)))
    return c * s

def _su2_cg(j1, j2, j3):
    mat = np.zeros((2 * j1 + 1, 2 * j2 + 1, 2 * j3 + 1), dtype=np.float64)
    for m1 in range(-j1, j1 + 1):
        for m2 in range(-j2, j2 + 1):
            m3 = m1 + m2
            if abs(m3) <= j3:
                mat[j1 + m1, j2 + m2, j3 + m3] = _su2_cg_coeff(j1, m1, j2, m2, j3, m3)
    return mat

def _change_basis_real_to_complex(l):
    q = np.zeros((2 * l + 1, 2 * l + 1), dtype=np.complex128)
    for m in range(-l, 0):
        q[l + m, l + abs(m)] = 1.0 / 2 ** 0.5
        q[l + m, l - abs(m)] = -1j / 2 ** 0.5
    q[l, l] = 1.0
    for m in range(1, l + 1):
        q[l + m, l + abs(m)] = (-1) ** m / 2 ** 0.5
        q[l + m, l - abs(m)] = 1j * (-1) ** m / 2 ** 0.5
    return (-1j) ** l * q

def _wigner_3j(l1, l2, l3):
    Q1 = _change_basis_real_to_complex(l1)
    Q2 = _change_basis_real_to_complex(l2)
    Q3 = _change_basis_real_to_complex(l3)
    Cc = _su2_cg(l1, l2, l3).astype(np.complex128)
    Cc = np.einsum('ij,kl,mn,ikn->jlm', Q1, Q2, np.conj(Q3.T), Cc)
    Cc = np.real(Cc)
    return (Cc / np.linalg.norm(Cc)).astype(np.float64)

_PATHS = [(4, 4, 4), (4, 4, 6), (4, 6, 4), (4, 6, 6),
          (6, 4, 4), (6, 4, 6), (6, 6, 4), (6, 6, 6)]
_W3J = {p: _wigner_3j(*p) for p in _PATHS}
_PW4 = math.sqrt(9.0 / 4.0)
_PW6 = math.sqrt(13.0 / 4.0)
_OFF = {4: 0, 6: 9}
_DIM = {4: 9, 6: 13}

def build_ct(tp_weights):
    Ct = np.zeros((22, 22, 22), np.float64)
    for p, (l1, l2, lo) in enumerate(_PATHS):
        w3 = _W3J[(l1, l2, lo)]
        scale = float(tp_weights[p]) * (_PW4 if lo == 4 else _PW6)
        Ct[_OFF[l1]:_OFF[l1]+_DIM[l1], _OFF[l2]:_OFF[l2]+_DIM[l2],
           _OFF[lo]:_OFF[lo]+_DIM[lo]] += scale * w3
    return Ct.astype(np.float32)

# upsample op list: (p_h, vtap, p_w, htap)
W_OPS = []
for _ph in range(4):
    for _vt in range(2):
        if _ph + 4 * _vt >= TP_K:
            continue
        for _pw in range(4):
            for _ht in range(2):
                if _pw + 4 * _ht >= TP_K:
                    continue
                W_OPS.append((_ph, _vt, _pw, _ht))
assert len(W_OPS) == 36

CTX_TAPS = [(dh, dw) for dh in (-1, 0, 1) for dw in (-1, 0, 1)]

# ---------------------------------------------------------------------------
# Bass program (built lazily, cached)
# ---------------------------------------------------------------------------
_CACHED_NC = None
LAST_EXEC_NS = None
TRACE = False

def build_nc():
    import concourse.bass as bass
    import concourse.bacc as bacc
    import concourse.tile as tile
    from concourse import mybir

    f32 = mybir.dt.float32
    bf16 = mybir.dt.bfloat16

    nc = bacc.Bacc(None)
    xin_d = nc.declare_dram_parameter("xin", [128, N_XIN_ROWS, N_XIN_COLS], f32, isOutput=False)
    cf_d = nc.declare_dram_parameter("constf", [128, 45], f32, isOutput=False)
    cb_d = nc.declare_dram_parameter("constb", [128, NCHUNK, 2 * ZROWS + 22], bf16, isOutput=False)
    out_d = nc.declare_dram_parameter("out", [22, 256, 512], f32, isOutput=True)

    with tile.TileContext(nc) as tc:
        with (
            tc.tile_pool(name="const", bufs=1) as constp,
            tc.tile_pool(name="feat", bufs=1) as featp,
            tc.tile_pool(name="ctx", bufs=1) as ctxp,
            tc.tile_pool(name="tmp", bufs=1) as tmpp,
            tc.tile_pool(name="z", bufs=3) as zp,
            tc.tile_pool(name="ysb", bufs=3) as ysbp,
            tc.tile_pool(name="orow", bufs=3) as orowp,
            tc.tile_pool(name="psx", bufs=2, space="PSUM") as psxp,
            tc.tile_pool(name="psy", bufs=2, space="PSUM") as psyp,
            tc.tile_pool(name="pso", bufs=2, space="PSUM") as psop,
        ):
            xin = constp.tile([128, N_XIN_ROWS, N_XIN_COLS], f32)
            nc.sync.dma_start(xin[:], xin_d[:])
            cf = constp.tile([128, 45], f32)
            nc.sync.dma_start(cf[:], cf_d[:])
            cb = constp.tile([128, NCHUNK, 2 * ZROWS + 22], bf16)
            nc.sync.dma_start(cb[:], cb_d[:])
            wv = cf[:, 0:36]
            sp9 = cf[:, 36:45]
            rx = cb[:, :, 0:ZROWS]
            ry = cb[:, :, ZROWS:2 * ZROWS]
            cck = cb[:, :, 2 * ZROWS:2 * ZROWS + 22]

            # absorb the input-DMA waits into single-wait DVE copies so that
            # later ops (esp. TensorScalarPtr, which has few sync-wait slots)
            # see the DMA ticks as already-observed on their engine
            w1 = tmpp.tile([128, 1], f32, tag="warm1")
            nc.vector.tensor_copy(w1[:], xin[:, 0, 0:1])
            w2 = tmpp.tile([128, 1], f32, tag="warm2")
            nc.vector.tensor_copy(w2[:], cf[:, 0:1])
            w3 = tmpp.tile([128, 1], bf16, tag="warm3")
            nc.vector.tensor_copy(w3[:], cb[:, 0, 0:1])

            for t in range(2):  # two 32-row chunks per quarter
                feat = featp.tile([128, 34, 516], bf16)
                # ---- upsample: group W_OPS by (p_h, p_w); first tap plain, rest via tmp+add
                groups = {}
                for oi, (ph, vt, pw, ht) in enumerate(W_OPS):
                    groups.setdefault((ph, pw), []).append((oi, vt, ht))
                for (ph, pw), taps in sorted(groups.items()):
                    # out rows rl = ph+4s (s=0..8, rl<34), cols cl = pw+1+4u (u=0..128, cl<516)
                    n_s = len([s for s in range(9) if ph + 4 * s < 34])
                    n_u = len([u for u in range(129) if pw + 1 + 4 * u < 516])
                    out_ap = feat[:, ph:ph + 4 * (n_s - 1) + 1:4, pw + 1:pw + 1 + 4 * (n_u - 1) + 1:4]
                    for k, (oi, vt, ht) in enumerate(taps):
                        rin0 = 8 * t + PAD_TOP - vt
                        cin0 = PAD_L - ht
                        in_ap = xin[:, rin0:rin0 + n_s, cin0:cin0 + n_u]
                        if k == 0:
                            nc.vector.tensor_scalar_mul(out_ap, in_ap, wv[:, oi:oi + 1])
                        else:
                            tmp = tmpp.tile([128, n_s, n_u], bf16, tag="uptmp")
                            nc.vector.tensor_scalar_mul(tmp[:], in_ap, wv[:, oi:oi + 1])
                            nc.vector.tensor_add(out_ap, out_ap, tmp[:])
                # ---- ctx 3x3
                ctx = ctxp.tile([128, 32, 512], bf16)
                for k, (dh, dw) in enumerate(CTX_TAPS):
                    in_ap = feat[:, 1 + dh:33 + dh, 2 + dw:514 + dw]
                    if k == 0:
                        nc.vector.tensor_scalar_mul(ctx[:], in_ap, sp9[:, k:k + 1])
                    else:
                        tmp = tmpp.tile([128, 32, 512], bf16, tag="ctxtmp")
                        nc.vector.tensor_scalar_mul(tmp[:], in_ap, sp9[:, k:k + 1])
                        nc.vector.tensor_add(ctx[:], ctx[:], tmp[:])
                # ---- TP per quarter per row
                for q in range(4):
                    for r in range(32):
                        x_ap = feat[32 * q:32 * q + 22, 1 + r, 2:514]   # [22, 512]
                        y_ap = ctx[32 * q:32 * q + 22, r, :]            # [22, 512]
                        out_ps = psop.tile([22, 512], f32)
                        for c in range(NCHUNK):
                            xs = psxp.tile([128, 512], f32)
                            ys = psyp.tile([128, 512], f32)
                            tp = (32 * q, 0)
                            nc.tensor.matmul(xs[:], rx[32 * q:32 * q + 22, c, :], x_ap,
                                             start=True, stop=True, tile_position=tp)
                            nc.tensor.matmul(ys[:], ry[32 * q:32 * q + 22, c, :], y_ap,
                                             start=True, stop=True, tile_position=tp)
                            ys_sb = ysbp.tile([128, 512], bf16)
                            nc.scalar.copy(ys_sb[:], ys[:])
                            z = zp.tile([128, 512], bf16)
                            nc.vector.tensor_mul(z[:], xs[:], ys_sb[:])
                            nc.tensor.matmul(out_ps[:], cck[:, c, :], z[:],
                                             start=(c == 0), stop=(c == NCHUNK - 1))
                        orow = orowp.tile([22, 512], f32)
                        nc.scalar.copy(orow[:], out_ps[:])
                        nc.sync.dma_start(out_d[:, 64 * q + 32 * t + r, :], orow[:])
    return nc

# ---------------------------------------------------------------------------
# Host-side input prep
# ---------------------------------------------------------------------------

def build_host_inputs(f4, f6, tconv_weight, spatial_weights, tp_weights):
    x_lr = np.concatenate([f4, f6], -1).reshape(B, H, W, C_FEAT).transpose(0, 3, 1, 2)
    x_lr = np.ascontiguousarray(x_lr, dtype=np.float32)

    # xin per core: [128, 19, 130] with zero padding and per-quarter shear
    xins = []
    for core in range(8):
        b, half = core // 2, core % 2
        r0 = 256 * half
        xin = np.zeros((128, N_XIN_ROWS, N_XIN_COLS), np.float32)
        for q in range(4):
            base = r0 // 4 + 16 * q
            lo = base - PAD_TOP
            src_lo = max(0, lo)
            src_hi = min(H, lo + N_XIN_ROWS)
            if src_hi > src_lo:
                dst_lo = src_lo - lo
                dst_hi = dst_lo + (src_hi - src_lo)
                xin[32 * q:32 * q + C_FEAT, dst_lo:dst_hi, PAD_L:PAD_L + W] = \
                    x_lr[b, :, src_lo:src_hi, :]
        xins.append(xin)

    # upsample weights per op
    wv = np.zeros((128, 36), np.float32)
    for oi, (ph, vt, pw, ht) in enumerate(W_OPS):
        val = tconv_weight[:, 0, ph + 4 * vt, pw + 4 * ht]  # (22,)
        for q in range(4):
            wv[32 * q:32 * q + C_FEAT, oi] = val

    sp9 = np.zeros((128, 9), np.float32)
    for k, (dh, dw) in enumerate(CTX_TAPS):
        sp9[:, k] = spatial_weights[dh + 1, dw + 1]

    # replication matrices and C chunks; z-row (c, m) -> pair p = c*128+m -> (i, j)
    Ct = build_ct(tp_weights)
    rxm = np.zeros((128, NCHUNK, ZROWS), np.float32)
    rym = np.zeros((128, NCHUNK, ZROWS), np.float32)
    ccm = np.zeros((128, NCHUNK, 22), np.float32)
    for c in range(NCHUNK):
        for m in range(ZROWS):
            p = c * ZROWS + m
            if p >= 484:
                continue
            i, j = p // 22, p % 22
            for q in range(4):
                rxm[32 * q + i, c, m] = 1.0
                rym[32 * q + j, c, m] = 1.0
            ccm[m, c, :] = Ct[i, j, :]
    return xins, wv, sp9, rxm.astype(BF16), rym.astype(BF16), ccm.astype(BF16), Ct

# ---------------------------------------------------------------------------
# Host-side border fix (exact fp32 for the 1-px border of each image)
# ---------------------------------------------------------------------------

def _upsample_rows(x_img, oh_list):
    # x_img: (22, H, W) fp32 -> feat rows (22, len(oh_list), Wr), zero-boundary
    out = np.zeros((C_FEAT, len(oh_list), Wr), np.float32)
    # precompute per-ow contributions
    for oidx, oh in enumerate(oh_list):
        acc = np.zeros((C_FEAT, Wr), np.float32)
        for kh in range(TP_K):
            num = oh + 1 - kh
            if num % 4 != 0:
                continue
            ih = num // 4
            if not (0 <= ih < H):
                continue
            row = x_img[:, ih, :]  # (22, W)
            for kw in range(TP_K):
                pw = (0 + 1 - kw) % 4  # ow congruence: ow+1 ≡ kw (mod 4)
                # ow = kw - 1 + 4*m, m >= 0
                ow0 = kw - 1
                ms = np.arange(W)
                ows = ow0 + 4 * ms
                sel = (ows >= 0) & (ows < Wr)
                iws = ms
                acc_cols = ows[sel]
                acc[:, acc_cols] += row[:, iws[sel]] * x_tconv[:, kh, kw][:, None]
        out[:, oidx, :] = acc
    return out

def _fix_borders(out_full_t, x_lr, tconv, spat, Ct):
    # out_full_t: (B, Hr, Wr, 22) fp32, modified in place on the 1-px border
    global x_tconv
    x_tconv = tconv[:, 0, :, :].astype(np.float32)
    for b in range(B):
        ximg = x_lr[b]
        border_rows = [0, Hr - 1]
        border_cols = [0, Wr - 1]
        # rows 0 and 511 need feat rows {-1->0,0,1} and {510,511,512->511}
        need_rows = sorted({0, 1, Hr - 2, Hr - 1})
        feat_rows = _upsample_rows(ximg, need_rows)  # (22, 4, Wr)
        fr = {oh: feat_rows[:, i, :] for i, oh in enumerate(need_rows)}
        # cols: need feat cols {0,1,510,511} full height -> compute feat full rows? too big.
        # Instead compute feat columns via transpose trick: upsample is separable per-tap in
        # our direct form; easiest: compute feat for all rows but only needed cols.
        # feat[:, oh, ow] for ow in {0,1,510,511}, all oh.
        feat_cols = np.zeros((C_FEAT, Hr, 4), np.float32)
        col_list = [0, 1, Wr - 2, Wr - 1]
        for kh in range(TP_K):
            pass
        # direct: for each needed ow, accumulate over (kh, kw)
        for ci, ow in enumerate(col_list):
            acc = np.zeros((C_FEAT, Hr), np.float32)
            for kw in range(TP_K):
                num = ow + 1 - kw
                if num % 4 != 0:
                    continue
                iw = num // 4
                if not (0 <= iw < W):
                    continue
                colv = ximg[:, :, iw]  # (22, H)
                for kh in range(TP_K):
                    oh0 = kh - 1
                    ms = np.arange(H)
                    ohs = oh0 + 4 * ms
                    sel = (ohs >= 0) & (ohs < Hr)
                    acc[:, ohs[sel]] += colv[:, ms[sel]] * x_tconv[:, kh, kw][:, None]
            feat_cols[:, :, ci] = acc
        # ctx + TP for border pixels
        def feat_at(oh, ow):
            ohc = min(max(oh, 0), Hr - 1)
            owc = min(max(ow, 0), Wr - 1)
            if ohc in fr:
                return fr[ohc][:, owc]
            if owc in col_list:
                return feat_cols[:, ohc, col_list.index(owc)]
            raise KeyError((oh, ow))
        border_px = ([(0, ow) for ow in range(Wr)] + [(Hr - 1, ow) for ow in range(Wr)]
                     + [(oh, 0) for oh in range(1, Hr - 1)] + [(oh, Wr - 1) for oh in range(1, Hr - 1)])
        xs = np.empty((len(border_px), C_FEAT), np.float32)
        ys = np.empty((len(border_px), C_FEAT), np.float32)
        for n, (oh, ow) in enumerate(border_px):
            xs[n] = feat_at(oh, ow)
            acc = np.zeros(C_FEAT, np.float32)
            for dh in (-1, 0, 1):
                for dw in (-1, 0, 1):
                    acc += spat[dh + 1, dw + 1] * feat_at(oh + dh, ow + dw)
            ys[n] = acc
        outs = np.einsum('ijk,ni,nj->nk', Ct, xs, ys)
        for n, (oh, ow) in enumerate(border_px):
            out_full_t[b, oh, ow, :] = outs[n]

# ---------------------------------------------------------------------------
# Entry point
# ---------------------------------------------------------------------------

def kernel(f4, f6, tconv_weight, spatial_weights, tp_weights, H=128, W=128):
    global _CACHED_NC
    from concourse.bass_utils import run_bass_kernel_spmd

    f4 = np.asarray(f4, np.float32)
    f6 = np.asarray(f6, np.float32)
    tconv_weight = np.asarray(tconv_weight, np.float32)
    spatial_weights = np.asarray(spatial_weights, np.float32)
    tp_weights = np.asarray(tp_weights, np.float32)

    xins, wv, sp9, rxm, rym, ccm, Ct = build_host_inputs(
        f4, f6, tconv_weight, spatial_weights, tp_weights)
    constf = np.concatenate([wv, sp9], axis=1).astype(np.float32)
    constb = np.concatenate([rxm, rym, ccm], axis=2).astype(BF16)

    if _CACHED_NC is None:
        nc_new = build_nc()
        nc_new.finalize()
        _CACHED_NC = nc_new
    nc = _CACHED_NC

    in_maps = []
    for core in range(8):
        in_maps.append({
            "xin": xins[core],
            "constf": constf,
            "constb": constb,
        })
    global LAST_EXEC_NS
    trace = bool(globals().get("TRACE", False)) or bool(os.environ.get("KERNEL_TRACE"))
    res = run_bass_kernel_spmd(nc, in_maps, list(range(8)), trace=trace)
    if res.exec_time_ns is not None:
        LAST_EXEC_NS = res.exec_time_ns
    outs = res.results

    # gather: per-core out [22, 256, 512] planar -> (B, Hr, Wr, 22)
    out_full = np.zeros((B, Hr, Wr, C_FEAT), np.float32)
    for core in range(8):
        b, half = core // 2, core % 2
        oc = np.asarray(outs[core]["out"])  # (22, 256, 512)
        out_full[b, 256 * half:256 * half + 256, :, :] = oc.transpose(1, 2, 0)

    x_lr = np.concatenate([f4, f6], -1).reshape(B, 128, 128, C_FEAT).transpose(0, 3, 1, 2)
    _fix_borders(out_full, np.ascontiguousarray(x_lr, np.float32),
                 tconv_weight, spatial_weights, Ct)

    out_t = out_full.reshape(B, Hr * Wr, C_FEAT)
    return out_t[..., :9].copy(), out_t[..., 9:].copy()


# revision 17
# speedup vs baseline: 1.0362x; 1.0362x over previous
# Trainium2 Bass kernel for nn_EquivariantTransposeConv.
# Self-contained: hardcodes shapes (B=4, H=W=128, R=4, C=22) and the sharding
# (8 cores, each core = half of one image = 256 HR rows).
#
# Per-core pipeline (all planar: channels on partitions, pixels on free dim):
#   1. host: pack LR input into a 4-quarter replicated, row-sheared planar
#      tensor xin[128, 19, 130] (quarter q rows 64q..64q+64 of the core's
#      256-row slab), plus small weight tensors derived from runtime inputs.
#   2. upsample (depthwise 6x6 stride-4 transpose conv) via per-(phase,tap)
#      tensor_scalar ops -> feat bf16 [128, 34, 516] per 32-row chunk.
#   3. ctx 3x3 depthwise conv via tensor_scalar/tensor_tensor -> ctx bf16.
#   4. per output row (512 px): replicate x/y channels to 484 product rows
#      with two 0/1 matmuls per 128-row chunk, multiply on VectorE, contract
#      with the folded Wigner tensor via 4 accumulating matmuls -> out[22,512].
#   5. host: gather per-core planar outputs, transpose to (B, N, 22), fix the
#      1-pixel border of each image exactly in fp32 numpy (the kernel computes
#      garbage there since edge-replication of ctx is not done on-device).
import math
import os
import sys
from fractions import Fraction

import numpy as np
import ml_dtypes

sys.path.insert(0, "/opt/trn_rl_repo")

C_FEAT = 22
R = 4
TP_K = 6
KS = 3
B, H, W = 4, 128, 128
Hr, Wr = H * R, W * R
PAD_TOP = 1
PAD_L = 1
N_XIN_ROWS = 19
N_XIN_COLS = 130
NCHUNK = 4          # z-row chunks
ZROWS = 128         # z-rows per chunk
BF16 = ml_dtypes.bfloat16

# ---------------------------------------------------------------------------
# Wigner 3j tables (identical math to the reference, self-contained copy)
# ---------------------------------------------------------------------------

def _fact(n):
    return Fraction(math.factorial(round(n)), 1)

def _su2_cg_coeff(j1, m1, j2, m2, j3, m3):
    if m3 != m1 + m2:
        return 0.0
    vmin = int(max(-j1 + j2 + m3, -j1 + m1, 0))
    vmax = int(min(j2 + j3 + m1, j3 - j1 + j2, j3 + m3))
    c = float((2.0 * j3 + 1.0) * Fraction(
        _fact(j3 + j1 - j2) * _fact(j3 - j1 + j2) * _fact(j1 + j2 - j3) * _fact(j3 + m3) * _fact(j3 - m3),
        _fact(j1 + j2 + j3 + 1) * _fact(j1 - m1) * _fact(j1 + m1) * _fact(j2 - m2) * _fact(j2 + m2))) ** 0.5
    s = 0.0
    for v in range(vmin, vmax + 1):
        s += (-1.0) ** int(v + j2 + m2) * float(Fraction(
            _fact(j2 + j3 + m1 - v) * _fact(j1 - m1 + v),
            _fact(v) * _fact(j3 - j1 + j2 - v) * _fact(j3 + m3 - v) * _fact(v + j1 - j2 - m3)))
    return c * s

def _su2_cg(j1, j2, j3):
    mat = np.zeros((2 * j1 + 1, 2 * j2 + 1, 2 * j3 + 1), dtype=np.float64)
    for m1 in range(-j1, j1 + 1):
        for m2 in range(-j2, j2 + 1):
            m3 = m1 + m2
            if abs(m3) <= j3:
                mat[j1 + m1, j2 + m2, j3 + m3] = _su2_cg_coeff(j1, m1, j2, m2, j3, m3)
    return mat

def _change_basis_real_to_complex(l):
    q = np.zeros((2 * l + 1, 2 * l + 1), dtype=np.complex128)
    for m in range(-l, 0):
        q[l + m, l + abs(m)] = 1.0 / 2 ** 0.5
        q[l + m, l - abs(m)] = -1j / 2 ** 0.5
    q[l, l] = 1.0
    for m in range(1, l + 1):
        q[l + m, l + abs(m)] = (-1) ** m / 2 ** 0.5
        q[l + m, l - abs(m)] = 1j * (-1) ** m / 2 ** 0.5
    return (-1j) ** l * q

def _wigner_3j(l1, l2, l3):
    Q1 = _change_basis_real_to_complex(l1)
    Q2 = _change_basis_real_to_complex(l2)
    Q3 = _change_basis_real_to_complex(l3)
    Cc = _su2_cg(l1, l2, l3).astype(np.complex128)
    Cc = np.einsum('ij,kl,mn,ikn->jlm', Q1, Q2, np.conj(Q3.T), Cc)
    Cc = np.real(Cc)
    return (Cc / np.linalg.norm(Cc)).astype(np.float64)

_PATHS = [(4, 4, 4), (4, 4, 6), (4, 6, 4), (4, 6, 6),
          (6, 4, 4), (6, 4, 6), (6, 6, 4), (6, 6, 6)]
_W3J = {p: _wigner_3j(*p) for p in _PATHS}
_PW4 = math.sqrt(9.0 / 4.0)
_PW6 = math.sqrt(13.0 / 4.0)
_OFF = {4: 0, 6: 9}
_DIM = {4: 9, 6: 13}

def build_ct(tp_weights):
    Ct = np.zeros((22, 22, 22), np.float64)
    for p, (l1, l2, lo) in enumerate(_PATHS):
        w3 = _W3J[(l1, l2, lo)]
        scale = float(tp_weights[p]) * (_PW4 if lo == 4 else _PW6)
        Ct[_OFF[l1]:_OFF[l1]+_DIM[l1], _OFF[l2]:_OFF[l2]+_DIM[l2],
           _OFF[lo]:_OFF[lo]+_DIM[lo]] += scale * w3
    return Ct.astype(np.float32)

# upsample op list: (p_h, vtap, p_w, htap)
W_OPS = []
for _ph in range(4):
    for _vt in range(2):
        if _ph + 4 * _vt >= TP_K:
            continue
        for _pw in range(4):
            for _ht in range(2):
                if _pw + 4 * _ht >= TP_K:
                    continue
                W_OPS.append((_ph, _vt, _pw, _ht))
assert len(W_OPS) == 36

CTX_TAPS = [(dh, dw) for dh in (-1, 0, 1) for dw in (-1, 0, 1)]

# ---------------------------------------------------------------------------
# Bass program (built lazily, cached)
# ---------------------------------------------------------------------------
_CACHED_NC = None
LAST_EXEC_NS = None
TRACE = False

def build_nc():
    import concourse.bass as bass
    import concourse.bacc as bacc
    import concourse.tile as tile
    from concourse import mybir

    f32 = mybir.dt.float32
    bf16 = mybir.dt.bfloat16

    nc = bacc.Bacc(None)
    xin_d = nc.declare_dram_parameter("xin", [128, N_XIN_ROWS, N_XIN_COLS], bf16, isOutput=False)
    cf_d = nc.declare_dram_parameter("constf", [128, 45], f32, isOutput=False)
    cb_d = nc.declare_dram_parameter("constb", [128, NCHUNK, 2 * ZROWS + 22], bf16, isOutput=False)
    out_d = nc.declare_dram_parameter("out", [22, 256, 512], f32, isOutput=True)

    with tile.TileContext(nc) as tc:
        with (
            tc.tile_pool(name="const", bufs=1) as constp,
            tc.tile_pool(name="feat", bufs=1) as featp,
            tc.tile_pool(name="ctx", bufs=1) as ctxp,
            tc.tile_pool(name="tmp", bufs=1) as tmpp,
            tc.tile_pool(name="z", bufs=4) as zp,
            tc.tile_pool(name="ysb", bufs=4) as ysbp,
            tc.tile_pool(name="orow", bufs=3) as orowp,
            tc.tile_pool(name="psx", bufs=2, space="PSUM") as psxp,
            tc.tile_pool(name="psy", bufs=2, space="PSUM") as psyp,
            tc.tile_pool(name="pso", bufs=2, space="PSUM") as psop,
        ):
            xin = constp.tile([128, N_XIN_ROWS, N_XIN_COLS], bf16)
            nc.sync.dma_start(xin[:], xin_d[:])
            cf = constp.tile([128, 45], f32)
            nc.sync.dma_start(cf[:], cf_d[:])
            cb = constp.tile([128, NCHUNK, 2 * ZROWS + 22], bf16)
            nc.sync.dma_start(cb[:], cb_d[:])
            wv = cf[:, 0:36]
            sp9 = cf[:, 36:45]
            rx = cb[:, :, 0:ZROWS]
            ry = cb[:, :, ZROWS:2 * ZROWS]
            cck = cb[:, :, 2 * ZROWS:2 * ZROWS + 22]

            # absorb the input-DMA waits into single-wait DVE copies so that
            # later ops (esp. TensorScalarPtr, which has few sync-wait slots)
            # see the DMA ticks as already-observed on their engine
            w1 = tmpp.tile([128, 1], f32, tag="warm1")
            nc.vector.tensor_copy(w1[:], xin[:, 0, 0:1])  # bf16
            w2 = tmpp.tile([128, 1], f32, tag="warm2")
            nc.vector.tensor_copy(w2[:], cf[:, 0:1])
            w3 = tmpp.tile([128, 1], bf16, tag="warm3")
            nc.vector.tensor_copy(w3[:], cb[:, 0, 0:1])

            for t in range(2):  # two 32-row chunks per quarter
                feat = featp.tile([128, 34, 516], bf16)
                # ---- upsample: group W_OPS by (p_h, p_w); first tap plain, rest via tmp+add
                groups = {}
                for oi, (ph, vt, pw, ht) in enumerate(W_OPS):
                    groups.setdefault((ph, pw), []).append((oi, vt, ht))
                for (ph, pw), taps in sorted(groups.items()):
                    # out rows rl = ph+4s (s=0..8, rl<34), cols cl = pw+1+4u (u=0..128, cl<516)
                    n_s = len([s for s in range(9) if ph + 4 * s < 34])
                    n_u = len([u for u in range(129) if pw + 1 + 4 * u < 516])
                    out_ap = feat[:, ph:ph + 4 * (n_s - 1) + 1:4, pw + 1:pw + 1 + 4 * (n_u - 1) + 1:4]
                    for k, (oi, vt, ht) in enumerate(taps):
                        rin0 = 8 * t + PAD_TOP - vt
                        cin0 = PAD_L - ht
                        in_ap = xin[:, rin0:rin0 + n_s, cin0:cin0 + n_u]
                        if k == 0:
                            nc.vector.tensor_scalar_mul(out_ap, in_ap, wv[:, oi:oi + 1])
                        else:
                            tmp = tmpp.tile([128, n_s, n_u], bf16, tag="uptmp")
                            nc.vector.tensor_scalar_mul(tmp[:], in_ap, wv[:, oi:oi + 1])
                            nc.vector.tensor_add(out_ap, out_ap, tmp[:])
                # ---- ctx 3x3
                ctx = ctxp.tile([128, 32, 512], bf16)
                for k, (dh, dw) in enumerate(CTX_TAPS):
                    in_ap = feat[:, 1 + dh:33 + dh, 2 + dw:514 + dw]
                    if k == 0:
                        nc.vector.tensor_scalar_mul(ctx[:], in_ap, sp9[:, k:k + 1])
                    else:
                        tmp = tmpp.tile([128, 32, 512], bf16, tag="ctxtmp")
                        nc.vector.tensor_scalar_mul(tmp[:], in_ap, sp9[:, k:k + 1])
                        nc.vector.tensor_add(ctx[:], ctx[:], tmp[:])
                # ---- TP per quarter per row
                for q in range(4):
                    for r in range(32):
                        x_ap = feat[32 * q:32 * q + 22, 1 + r, 2:514]   # [22, 512]
                        y_ap = ctx[32 * q:32 * q + 22, r, :]            # [22, 512]
                        out_ps = psop.tile([22, 512], f32)
                        for c in range(NCHUNK):
                            xs = psxp.tile([128, 512], f32)
                            ys = psyp.tile([128, 512], f32)
                            tp = (32 * q, 0)
                            nc.tensor.matmul(xs[:], rx[32 * q:32 * q + 22, c, :], x_ap,
                                             start=True, stop=True, tile_position=tp)
                            nc.tensor.matmul(ys[:], ry[32 * q:32 * q + 22, c, :], y_ap,
                                             start=True, stop=True, tile_position=tp)
                            ys_sb = ysbp.tile([128, 512], bf16)
                            nc.scalar.copy(ys_sb[:], ys[:])
                            z = zp.tile([128, 512], bf16)
                            nc.vector.tensor_mul(z[:], xs[:], ys_sb[:])
                            nc.tensor.matmul(out_ps[:], cck[:, c, :], z[:],
                                             start=(c == 0), stop=(c == NCHUNK - 1))
                        orow = orowp.tile([22, 512], f32)
                        nc.scalar.copy(orow[:], out_ps[:])
                        nc.sync.dma_start(out_d[:, 64 * q + 32 * t + r, :], orow[:])
    return nc

# ---------------------------------------------------------------------------
# Host-side input prep
# ---------------------------------------------------------------------------

def build_host_inputs(f4, f6, tconv_weight, spatial_weights, tp_weights):
    x_lr = np.concatenate([f4, f6], -1).reshape(B, H, W, C_FEAT).transpose(0, 3, 1, 2)
    x_lr = np.ascontiguousarray(x_lr, dtype=np.float32)

    # xin per core: [128, 19, 130] with zero padding and per-quarter shear
    xins = []
    for core in range(8):
        b, half = core // 2, core % 2
        r0 = 256 * half
        xin = np.zeros((128, N_XIN_ROWS, N_XIN_COLS), np.float32)
        for q in range(4):
            base = r0 // 4 + 16 * q
            lo = base - PAD_TOP
            src_lo = max(0, lo)
            src_hi = min(H, lo + N_XIN_ROWS)
            if src_hi > src_lo:
                dst_lo = src_lo - lo
                dst_hi = dst_lo + (src_hi - src_lo)
                xin[32 * q:32 * q + C_FEAT, dst_lo:dst_hi, PAD_L:PAD_L + W] = \
                    x_lr[b, :, src_lo:src_hi, :]
        xins.append(xin.astype(BF16))

    # upsample weights per op
    wv = np.zeros((128, 36), np.float32)
    for oi, (ph, vt, pw, ht) in enumerate(W_OPS):
        val = tconv_weight[:, 0, ph + 4 * vt, pw + 4 * ht]  # (22,)
        for q in range(4):
            wv[32 * q:32 * q + C_FEAT, oi] = val

    sp9 = np.zeros((128, 9), np.float32)
    for k, (dh, dw) in enumerate(CTX_TAPS):
        sp9[:, k] = spatial_weights[dh + 1, dw + 1]

    # replication matrices and C chunks; z-row (c, m) -> pair p = c*128+m -> (i, j)
    Ct = build_ct(tp_weights)
    rxm = np.zeros((128, NCHUNK, ZROWS), np.float32)
    rym = np.zeros((128, NCHUNK, ZROWS), np.float32)
    ccm = np.zeros((128, NCHUNK, 22), np.float32)
    for c in range(NCHUNK):
        for m in range(ZROWS):
            p = c * ZROWS + m
            if p >= 484:
                continue
            i, j = p // 22, p % 22
            for q in range(4):
                rxm[32 * q + i, c, m] = 1.0
                rym[32 * q + j, c, m] = 1.0
            ccm[m, c, :] = Ct[i, j, :]
    return xins, wv, sp9, rxm.astype(BF16), rym.astype(BF16), ccm.astype(BF16), Ct

# ---------------------------------------------------------------------------
# Host-side border fix (exact fp32 for the 1-px border of each image)
# ---------------------------------------------------------------------------

def _upsample_rows(x_img, oh_list):
    # x_img: (22, H, W) fp32 -> feat rows (22, len(oh_list), Wr), zero-boundary
    out = np.zeros((C_FEAT, len(oh_list), Wr), np.float32)
    # precompute per-ow contributions
    for oidx, oh in enumerate(oh_list):
        acc = np.zeros((C_FEAT, Wr), np.float32)
        for kh in range(TP_K):
            num = oh + 1 - kh
            if num % 4 != 0:
                continue
            ih = num // 4
            if not (0 <= ih < H):
                continue
            row = x_img[:, ih, :]  # (22, W)
            for kw in range(TP_K):
                pw = (0 + 1 - kw) % 4  # ow congruence: ow+1 ≡ kw (mod 4)
                # ow = kw - 1 + 4*m, m >= 0
                ow0 = kw - 1
                ms = np.arange(W)
                ows = ow0 + 4 * ms
                sel = (ows >= 0) & (ows < Wr)
                iws = ms
                acc_cols = ows[sel]
                acc[:, acc_cols] += row[:, iws[sel]] * x_tconv[:, kh, kw][:, None]
        out[:, oidx, :] = acc
    return out

def _fix_borders(out_full_t, x_lr, tconv, spat, Ct):
    # out_full_t: (B, Hr, Wr, 22) fp32, modified in place on the 1-px border
    global x_tconv
    x_tconv = tconv[:, 0, :, :].astype(np.float32)
    for b in range(B):
        ximg = x_lr[b]
        border_rows = [0, Hr - 1]
        border_cols = [0, Wr - 1]
        # rows 0 and 511 need feat rows {-1->0,0,1} and {510,511,512->511}
        need_rows = sorted({0, 1, Hr - 2, Hr - 1})
        feat_rows = _upsample_rows(ximg, need_rows)  # (22, 4, Wr)
        fr = {oh: feat_rows[:, i, :] for i, oh in enumerate(need_rows)}
        # cols: need feat cols {0,1,510,511} full height -> compute feat full rows? too big.
        # Instead compute feat columns via transpose trick: upsample is separable per-tap in
        # our direct form; easiest: compute feat for all rows but only needed cols.
        # feat[:, oh, ow] for ow in {0,1,510,511}, all oh.
        feat_cols = np.zeros((C_FEAT, Hr, 4), np.float32)
        col_list = [0, 1, Wr - 2, Wr - 1]
        for kh in range(TP_K):
            pass
        # direct: for each needed ow, accumulate over (kh, kw)
        for ci, ow in enumerate(col_list):
            acc = np.zeros((C_FEAT, Hr), np.float32)
            for kw in range(TP_K):
                num = ow + 1 - kw
                if num % 4 != 0:
                    continue
                iw = num // 4
                if not (0 <= iw < W):
                    continue
                colv = ximg[:, :, iw]  # (22, H)
                for kh in range(TP_K):
                    oh0 = kh - 1
                    ms = np.arange(H)
                    ohs = oh0 + 4 * ms
                    sel = (ohs >= 0) & (ohs < Hr)
                    acc[:, ohs[sel]] += colv[:, ms[sel]] * x_tconv[:, kh, kw][:, None]
            feat_cols[:, :, ci] = acc
        # ctx + TP for border pixels
        def feat_at(oh, ow):
            ohc = min(max(oh, 0), Hr - 1)
            owc = min(max(ow, 0), Wr - 1)
            if ohc in fr:
                return fr[ohc][:, owc]
            if owc in col_list:
                return feat_cols[:, ohc, col_list.index(owc)]
            raise KeyError((oh, ow))
        border_px = ([(0, ow) for ow in range(Wr)] + [(Hr - 1, ow) for ow in range(Wr)]
                     + [(oh, 0) for oh in range(1, Hr - 1)] + [(oh, Wr - 1) for oh in range(1, Hr - 1)])
        xs = np.empty((len(border_px), C_FEAT), np.float32)
        ys = np.empty((len(border_px), C_FEAT), np.float32)
        for n, (oh, ow) in enumerate(border_px):
            xs[n] = feat_at(oh, ow)
            acc = np.zeros(C_FEAT, np.float32)
            for dh in (-1, 0, 1):
                for dw in (-1, 0, 1):
                    acc += spat[dh + 1, dw + 1] * feat_at(oh + dh, ow + dw)
            ys[n] = acc
        outs = np.einsum('ijk,ni,nj->nk', Ct, xs, ys)
        for n, (oh, ow) in enumerate(border_px):
            out_full_t[b, oh, ow, :] = outs[n]

# ---------------------------------------------------------------------------
# Entry point
# ---------------------------------------------------------------------------

def kernel(f4, f6, tconv_weight, spatial_weights, tp_weights, H=128, W=128):
    global _CACHED_NC
    from concourse.bass_utils import run_bass_kernel_spmd

    f4 = np.asarray(f4, np.float32)
    f6 = np.asarray(f6, np.float32)
    tconv_weight = np.asarray(tconv_weight, np.float32)
    spatial_weights = np.asarray(spatial_weights, np.float32)
    tp_weights = np.asarray(tp_weights, np.float32)

    xins, wv, sp9, rxm, rym, ccm, Ct = build_host_inputs(
        f4, f6, tconv_weight, spatial_weights, tp_weights)
    constf = np.concatenate([wv, sp9], axis=1).astype(np.float32)
    constb = np.concatenate([rxm, rym, ccm], axis=2).astype(BF16)

    if _CACHED_NC is None:
        nc_new = build_nc()
        nc_new.finalize()
        _CACHED_NC = nc_new
    nc = _CACHED_NC

    in_maps = []
    for core in range(8):
        in_maps.append({
            "xin": xins[core],
            "constf": constf,
            "constb": constb,
        })
    global LAST_EXEC_NS
    trace = bool(globals().get("TRACE", False)) or bool(os.environ.get("KERNEL_TRACE"))
    res = run_bass_kernel_spmd(nc, in_maps, list(range(8)), trace=trace)
    if res.exec_time_ns is not None:
        LAST_EXEC_NS = res.exec_time_ns
    outs = res.results

    # gather: per-core out [22, 256, 512] planar -> (B, Hr, Wr, 22)
    out_full = np.zeros((B, Hr, Wr, C_FEAT), np.float32)
    for core in range(8):
        b, half = core // 2, core % 2
        oc = np.asarray(outs[core]["out"])  # (22, 256, 512)
        out_full[b, 256 * half:256 * half + 256, :, :] = oc.transpose(1, 2, 0)

    x_lr = np.concatenate([f4, f6], -1).reshape(B, 128, 128, C_FEAT).transpose(0, 3, 1, 2)
    _fix_borders(out_full, np.ascontiguousarray(x_lr, np.float32),
                 tconv_weight, spatial_weights, Ct)

    out_t = out_full.reshape(B, Hr * Wr, C_FEAT)
    return out_t[..., :9].copy(), out_t[..., 9:].copy()
